# revision 6
# baseline (speedup 1.0000x reference)
"""Trainium2 Bass kernel for nn_Encoder_Postnet (length-regulator gather + per-frame linears).

Contract: kernel(**inputs) takes FULL numpy inputs (as produced by
setup_inputs) and returns the FULL [B, T, H] float32 output. Internally the
batch dim is sharded across 8 NeuronCores (pure data parallel, 4 batches per
core); the tiny Linear(1,H) params are replicated.

v3 design: ONE DoubleRow fp8 matmul per 128-frame chunk computes BOTH the
length-regulator gather and the rank-update linears; ~20.5 MB HBM/core.

  - Per chunk, the encoder rows needed span <= 2*K_WIN consecutive indices
    (idx increments by at most 1 per frame). The host materializes a pair-
    packed window: partition k of the rhs holds rows [w+2k | w+2k+1] (1 KB),
    and 11 extra partitions hold the 22 rank-update rows as pairs.
    DoubleRow matmul semantics (out = lhsT[:,0].T @ rhs[:,0] +
    lhsT[:,1].T @ rhs[:,1], fp8, 2x throughput) then give
      out[m,:] = sum_k S_even[k,m]*row_{w+2k} + S_odd[k,m]*row_{w+2k+1}
               + sum_j A_j[m]*W_j
    with the one-hot S and the A rows host-built in the lhsT. No on-device
    scan, no SWDGE gather (34us of Q7 descriptor-gen on v2), no identity
    matmuls, and PE streams each chunk once at 2 cols/cycle.
  - rank rows (22): pos*w_pos via exact base-8 digit split of t
    (t = sum a_q 8^q, rows a_q*c0_q x c1_q*w_s with c0*c1 = 8^q, all values
    e4m3-exact) against a 3-way e4m3 split of w_pos (residual ~1e-3 rel);
    pitch/beats/bias via hi/lo e4m3 splits.
  - instruction stream is identical across the 8 cores (SPMD NEFF): the
    window base w and one-hots live in the DATA; only the single constant
    K_WIN (from the max chunk span) parameterizes the compiled kernel.
  - finishers: PSUM -> SBUF bf16 copies alternate Scalar/Vector engines
    (GPSIMD has no PSUM port); output written in BF16 (16.8 MB vs 33.5 f32)
    and upcast on the host. Total rel err ~2e-3 vs the 2e-2 gate.
"""

import sys

if "/opt/trn_rl_repo" not in sys.path:
    sys.path.insert(0, "/opt/trn_rl_repo")

from contextlib import ExitStack

import numpy as np

import concourse.bass as bass
import concourse.tile as tile
from concourse import bacc, mybir
from concourse.bass_utils import run_bass_kernel_spmd

B, T, P, H = 32, 4096, 512, 512
NCORES = 8
BPC = B // NCORES            # batches per core
CH = 128                     # frames per chunk (partition dim)
NCHUNK = T // CH             # 32 chunks per batch
GRP = 4                      # chunks per finisher group (4 psum banks)
GRP_T = GRP * CH
NG = NCHUNK // GRP           # 8 groups per batch
K_RANK = 11                  # rank-update row PAIRS (22 rows)
F32 = mybir.dt.float32
BF16 = mybir.dt.bfloat16
FP8 = mybir.dt.float8e4
DR = mybir.MatmulPerfMode.DoubleRow

# pos = t*w_pos with t = sum_q a_q 8^q split as (a_q*c0_q)*(c1_q*w_s):
# every factor exactly representable in e4m3 (<=112 / <=32).
C0 = [1.0, 2.0, 8.0, 16.0]
C1 = [1.0, 4.0, 8.0, 32.0]


def _emit(ctx: ExitStack, tc: tile.TileContext, kwin, ewin, smat, out):
    nc = tc.nc
    KK = kwin + K_RANK
    const = ctx.enter_context(tc.tile_pool(name="const", bufs=1))
    epool = ctx.enter_context(tc.tile_pool(name="epool", bufs=2))
    opool = ctx.enter_context(tc.tile_pool(name="opool", bufs=6))
    ppool = ctx.enter_context(tc.tile_pool(name="ppool", bufs=2, space="PSUM"))

    # all smat resident (8 KB/partition); ewin rotates 2 batches (32 KB each).
    sm = const.tile([KK, BPC * NCHUNK * 2 * CH], FP8)
    nc.sync.dma_start(sm[:], smat[:])

    ews = {}

    def load_ewin(b):
        ew = epool.tile([KK, NCHUNK * 2 * H], FP8, name=f"ew{b}", tag="ew")
        # issue on the (otherwise idle) gpsimd ring so batch b+1's load can
        # start while the sync ring is busy with output writes
        nc.gpsimd.dma_start(ew[:], ewin[:, b * NCHUNK * 2 * H:
                                        (b + 1) * NCHUNK * 2 * H])
        ews[b] = ew

    load_ewin(0)
    load_ewin(1)

    for b in range(BPC):
        if b + 2 < BPC:
            load_ewin(b + 2)
        ew = ews[b]
        for g in range(NG):
            gi = b * NG + g
            ps = ppool.tile([128, GRP * H], F32)
            for j in range(GRP):
                cl = g * GRP + j
                ci = b * NCHUNK + cl
                nc.tensor.matmul(
                    ps[:, j * H:(j + 1) * H],
                    lhsT=sm[:, ci * 2 * CH:(ci + 1) * 2 * CH].rearrange(
                        "p (two m) -> p two m", two=2),
                    rhs=ew[:, cl * 2 * H:(cl + 1) * 2 * H].rearrange(
                        "p (two n) -> p two n", two=2),
                    start=True, stop=True, perf_mode=DR)
            ot = opool.tile([128, GRP * H], BF16)
            if gi % 2 == 0:
                nc.scalar.copy(ot[:], ps[:])
            else:
                nc.vector.tensor_copy(ot[:], ps[:])
            nc.sync.dma_start(
                out[b * T + g * GRP_T: b * T + (g + 1) * GRP_T, :].rearrange(
                    "(j p) h -> p j h", p=128),
                ot[:].rearrange("p (j h) -> p j h", h=H))


_CACHED = {}


def _build(kwin):
    if kwin in _CACHED:
        return _CACHED[kwin]
    KK = kwin + K_RANK
    nc = bacc.Bacc("TRN2", target_bir_lowering=False, debug=False)
    ewin = nc.dram_tensor("ewin", (KK, BPC * NCHUNK * 2 * H), FP8,
                          kind="ExternalInput").ap()
    smat = nc.dram_tensor("smat", (KK, BPC * NCHUNK * 2 * CH), FP8,
                          kind="ExternalInput").ap()
    out = nc.dram_tensor("out", (BPC * T, H), BF16, kind="ExternalOutput").ap()

    with tile.TileContext(nc) as tc:
        with ExitStack() as ctx:
            _emit(ctx, tc, kwin, ewin, smat, out)
    nc.compile()
    _CACHED[kwin] = nc
    return nc


def _host_prep(encoder_out, pitch, beats, align_phone,
               w_pitch, b_pitch, w_beats, b_beats, w_pos, b_pos):
    """Compute idx on the host; build rank rows, per-chunk windows and
    one-hot/rank lhsT matrices. Returns (kwin, in_maps)."""
    import ml_dtypes
    fp8 = ml_dtypes.float8_e4m3
    f32 = np.float32

    align = np.asarray(align_phone, np.int32)
    change = np.concatenate(
        [np.zeros((B, 1), np.int32),
         (align[:, 1:] != align[:, :-1]).astype(np.int32)], axis=1)
    idx = np.clip(np.cumsum(change, axis=1), 0, P - 1)    # [B, T]

    # chunk spans -> window size (uniform across cores; baked into the NEFF)
    idx_c = idx.reshape(B, NCHUNK, CH)
    lo = idx_c[:, :, 0]                                    # [B, NCHUNK]
    hi = idx_c[:, :, -1]
    span = int((hi - lo + 1).max())
    kwin = max(8, -(-span // 2) + 1)
    kwin = -(-kwin // 4) * 4                               # round up to /4
    assert span <= 2 * kwin
    KK = kwin + K_RANK
    wbase = np.minimum(lo, P - 2 * kwin)                   # [B, NCHUNK]

    # 22 rank rows: lhsT values AR [22, T] (pos digits shared, pitch/beats
    # per batch) and rhs values WR [22, H]
    def e4(x):
        return np.asarray(x, f32).astype(fp8).astype(f32)

    def split3(w):
        w = np.asarray(w, f32)
        s0 = e4(w)
        s1 = e4(w - s0)
        s2 = e4(w - s0 - s1)
        return s0, s1, s2

    def split2(w):
        w = np.asarray(w, f32)
        s0 = e4(w)
        s1 = e4(w - s0)
        return s0, s1

    t = np.arange(T, dtype=np.int64)
    digits = [((t // (8 ** q)) % 8).astype(f32) for q in range(4)]
    ws = split3(w_pos)
    wp = split2(w_pitch)
    wb = split2(w_beats)
    bs = split2(np.asarray(b_pitch, f32) + np.asarray(b_beats, f32)
                + np.asarray(b_pos, f32))
    pit = split2(pitch)                                    # [2][B, T]
    bea = split2(beats)

    WR = np.zeros((22, H), f32)
    AR_shared = np.zeros((22, T), f32)                     # rows 0..11, 20..21
    for q in range(4):
        for s in range(3):
            r = q * 3 + s
            AR_shared[r] = digits[q] * C0[q]
            WR[r] = C1[q] * ws[s]
    WR[12], WR[13], WR[14], WR[15] = wp[0], wp[1], wp[0], wp[1]
    WR[16], WR[17], WR[18], WR[19] = wb[0], wb[1], wb[0], wb[1]
    AR_shared[20] = 1.0
    AR_shared[21] = 1.0
    WR[20], WR[21] = bs[0], bs[1]

    enc = np.ascontiguousarray(encoder_out, f32)           # [B, P, H]

    in_maps = []
    for r in range(NCORES):
        ewin_np = np.zeros((KK, BPC * NCHUNK * 2 * H), f32)
        smat_np = np.zeros((KK, BPC * NCHUNK * 2 * CH), f32)
        for bi in range(BPC):
            bg = r * BPC + bi
            AR = AR_shared.copy()
            AR[12], AR[13] = pit[0][bg], pit[0][bg]
            AR[14], AR[15] = pit[1][bg], pit[1][bg]
            AR[16], AR[17] = bea[0][bg], bea[0][bg]
            AR[18], AR[19] = bea[1][bg], bea[1][bg]
            for cl in range(NCHUNK):
                ci = bi * NCHUNK + cl
                w = int(wbase[bg, cl])
                # rhs: window row pairs + rank row pairs
                rows = enc[bg, w:w + 2 * kwin].reshape(kwin, 2 * H)
                ewin_np[:kwin, ci * 2 * H:(ci + 1) * 2 * H] = rows
                ewin_np[kwin:, ci * 2 * H:(ci + 1) * 2 * H] = \
                    WR.reshape(K_RANK, 2 * H)
                # lhsT: one-hot halves + rank rows
                rel = idx[bg, cl * CH:(cl + 1) * CH] - w   # [CH] in [0,2kwin)
                sblk = np.zeros((kwin, 2, CH), f32)
                sblk[rel // 2, rel % 2, np.arange(CH)] = 1.0
                smat_np[:kwin, ci * 2 * CH:(ci + 1) * 2 * CH] = \
                    sblk.reshape(kwin, 2 * CH)
                ablk = AR[:, cl * CH:(cl + 1) * CH].reshape(K_RANK, 2, CH)
                smat_np[kwin:, ci * 2 * CH:(ci + 1) * 2 * CH] = \
                    ablk.reshape(K_RANK, 2 * CH)
        in_maps.append({
            "ewin": ewin_np.astype(fp8),
            "smat": smat_np.astype(fp8),
        })
    return kwin, in_maps


def _run_in_subprocess(kwargs):
    """Fallback for a wedged in-process PJRT client: re-run this module in a
    fresh interpreter (fresh device boot), passing inputs via pickle."""
    import os
    import pickle
    import subprocess
    import tempfile

    with tempfile.TemporaryDirectory() as td:
        inp = os.path.join(td, "in.pkl")
        outp = os.path.join(td, "out.npy")
        with open(inp, "wb") as f:
            pickle.dump(kwargs, f)
        code = (
            "import pickle, numpy as np, importlib.util\n"
            f"spec = importlib.util.spec_from_file_location('k', {__file__!r})\n"
            "m = importlib.util.module_from_spec(spec)\n"
            f"ins = pickle.load(open({inp!r}, 'rb'))\n"
            "spec.loader.exec_module(m)\n"
            f"np.save({outp!r}, m.kernel(**ins, _no_fallback=True))\n"
        )
        subprocess.run([sys.executable, "-c", code], check=True, timeout=1700)
        return np.load(outp)


def kernel(encoder_out, pitch, beats, w_pitch, b_pitch, w_beats, b_beats,
           w_pos, b_pos, align_phone, _trace=False, _no_fallback=False):
    kwargs = dict(encoder_out=np.asarray(encoder_out),
                  pitch=np.asarray(pitch), beats=np.asarray(beats),
                  w_pitch=np.asarray(w_pitch), b_pitch=np.asarray(b_pitch),
                  w_beats=np.asarray(w_beats), b_beats=np.asarray(b_beats),
                  w_pos=np.asarray(w_pos), b_pos=np.asarray(b_pos),
                  align_phone=np.asarray(align_phone))
    kwin, in_maps = _host_prep(
        encoder_out, pitch, beats, align_phone,
        w_pitch, b_pitch, w_beats, b_beats, w_pos, b_pos)
    nc = _build(kwin)

    def attempt():
        # materialize eagerly so device failures surface inside the guard
        res = run_bass_kernel_spmd(nc, in_maps, core_ids=list(range(NCORES)),
                                   trace=_trace)
        return res, np.concatenate(
            [np.asarray(res.results[r]["out"]).astype(np.float32).reshape(
                BPC, T, H) for r in range(NCORES)], axis=0)

    import time
    res = out = None
    for i in range(2):
        try:
            res, out = attempt()
            break
        except Exception:
            # rare flaky device hang (NRT_EXEC_UNIT_UNRECOVERABLE)
            time.sleep(5.0)
    if out is None:
        if _no_fallback:
            res, out = attempt()
        else:
            # fresh interpreter = fresh PJRT client + device reset
            try:
                return _run_in_subprocess(kwargs)
            except Exception:
                time.sleep(10.0)
                return _run_in_subprocess(kwargs)
    if _trace:
        kernel.last_results = res
    return out


# revision 12
# speedup vs baseline: 1.1160x; 1.1160x over previous
"""Trainium2 Bass kernel for nn_Encoder_Postnet (length-regulator gather + per-frame linears).

Contract: kernel(**inputs) takes FULL numpy inputs (as produced by
setup_inputs) and returns the FULL [B, T, H] float32 output. Internally the
batch dim is sharded across 8 NeuronCores (pure data parallel, 4 batches per
core); the tiny Linear(1,H) params are replicated.

v3 design: ONE DoubleRow fp8 matmul per 128-frame chunk computes BOTH the
length-regulator gather and the rank-update linears; ~20.5 MB HBM/core.

  - Per chunk, the encoder rows needed span <= 2*K_WIN consecutive indices
    (idx increments by at most 1 per frame). The host materializes a pair-
    packed window: partition k of the rhs holds rows [w+2k | w+2k+1] (1 KB),
    and 11 extra partitions hold the 22 rank-update rows as pairs.
    DoubleRow matmul semantics (out = lhsT[:,0].T @ rhs[:,0] +
    lhsT[:,1].T @ rhs[:,1], fp8, 2x throughput) then give
      out[m,:] = sum_k S_even[k,m]*row_{w+2k} + S_odd[k,m]*row_{w+2k+1}
               + sum_j A_j[m]*W_j
    with the one-hot S and the A rows host-built in the lhsT. No on-device
    scan, no SWDGE gather (34us of Q7 descriptor-gen on v2), no identity
    matmuls, and PE streams each chunk once at 2 cols/cycle.
  - rank rows (22): pos*w_pos via exact base-8 digit split of t
    (t = sum a_q 8^q, rows a_q*c0_q x c1_q*w_s with c0*c1 = 8^q, all values
    e4m3-exact) against a 3-way e4m3 split of w_pos (residual ~1e-3 rel);
    pitch/beats/bias via hi/lo e4m3 splits.
  - instruction stream is identical across the 8 cores (SPMD NEFF): the
    window base w and one-hots live in the DATA; only the single constant
    K_WIN (from the max chunk span) parameterizes the compiled kernel.
  - finishers: PSUM -> SBUF bf16 copies alternate Scalar/Vector engines
    (GPSIMD has no PSUM port); output written in BF16 (16.8 MB vs 33.5 f32)
    and upcast on the host. Total rel err ~2e-3 vs the 2e-2 gate.
"""

import sys

if "/opt/trn_rl_repo" not in sys.path:
    sys.path.insert(0, "/opt/trn_rl_repo")

from contextlib import ExitStack

import numpy as np

import concourse.bass as bass
import concourse.tile as tile
from concourse import bacc, mybir
from concourse.bass_utils import run_bass_kernel_spmd

B, T, P, H = 32, 4096, 512, 512
NCORES = 8
BPC = B // NCORES            # batches per core
CH = 128                     # frames per chunk (partition dim)
NCHUNK = T // CH             # 32 chunks per batch
GRP = 4                      # chunks per finisher group (4 psum banks)
GRP_T = GRP * CH
NG = NCHUNK // GRP           # 8 groups per batch
K_RANK = 11                  # rank-update row PAIRS (22 rows)
SEGB = 2048                  # input-load DMA segment bytes (descriptor size)
F32 = mybir.dt.float32
BF16 = mybir.dt.bfloat16
FP8 = mybir.dt.float8e4
DR = mybir.MatmulPerfMode.DoubleRow

# pos = t*w_pos with t = sum_q a_q 8^q split as (a_q*c0_q)*(c1_q*w_s):
# every factor exactly representable in e4m3 (<=112 / <=32).
C0 = [1.0, 2.0, 8.0, 16.0]
C1 = [1.0, 4.0, 8.0, 32.0]


def _emit(ctx: ExitStack, tc: tile.TileContext, kwin, ewin, smat, out):
    nc = tc.nc
    KK = kwin + K_RANK
    const = ctx.enter_context(tc.tile_pool(name="const", bufs=1))
    epool = ctx.enter_context(tc.tile_pool(name="epool", bufs=2))
    opool = ctx.enter_context(tc.tile_pool(name="opool", bufs=6))
    ppool = ctx.enter_context(tc.tile_pool(name="ppool", bufs=2, space="PSUM"))

    # Input DMAs to an SBUF dst emit ONE descriptor per dst partition, and a
    # KK(=23)-partition load with 32KB runs crawls at 1-2 DMA engines (~62us
    # observed). The DRAM side is therefore pre-interleaved in 2KB segments
    # (row (b*NSEG+s)*KK+p holds partition p's bytes [2048s, 2048(s+1))), so
    # each load becomes KK*NSEG small descriptors spread over all 16 engines.
    EB = NCHUNK * 2 * H          # ewin bytes/partition/batch (32K)
    SB_ = NCHUNK * 2 * CH        # smat bytes/partition/batch (8K)
    sms = []
    for b in range(BPC):
        sm = const.tile([KK, SB_], FP8, tag=f"sm{b}", name=f"sm{b}")
        nc.sync.dma_start(
            sm[:], smat[b * (SB_ // SEGB) * KK:(b + 1) * (SB_ // SEGB) * KK,
                        :].rearrange("(s p) x -> p s x", p=KK))
        sms.append(sm)

    ews = {}

    def load_ewin(b):
        ew = epool.tile([KK, EB], FP8, name=f"ew{b}", tag="ew")
        # issue on the (otherwise idle) gpsimd ring so batch b+1's load can
        # start while the sync ring is busy with output writes
        nc.gpsimd.dma_start(
            ew[:], ewin[b * (EB // SEGB) * KK:(b + 1) * (EB // SEGB) * KK,
                        :].rearrange("(s p) x -> p s x", p=KK))
        ews[b] = ew

    load_ewin(0)
    load_ewin(1)

    for b in range(BPC):
        if b + 2 < BPC:
            load_ewin(b + 2)
        ew = ews[b]
        sm = sms[b]
        for g in range(NG):
            gi = b * NG + g
            ps = ppool.tile([128, GRP * H], F32)
            for j in range(GRP):
                cl = g * GRP + j
                nc.tensor.matmul(
                    ps[:, j * H:(j + 1) * H],
                    lhsT=sm[:, cl * 2 * CH:(cl + 1) * 2 * CH].rearrange(
                        "p (two m) -> p two m", two=2),
                    rhs=ew[:, cl * 2 * H:(cl + 1) * 2 * H].rearrange(
                        "p (two n) -> p two n", two=2),
                    start=True, stop=True, perf_mode=DR)
            ot = opool.tile([128, GRP * H], BF16)
            if gi % 2 == 0:
                nc.scalar.copy(ot[:], ps[:])
            else:
                nc.vector.tensor_copy(ot[:], ps[:])
            nc.sync.dma_start(
                out[b * T + g * GRP_T: b * T + (g + 1) * GRP_T, :].rearrange(
                    "(j p) h -> p j h", p=128),
                ot[:].rearrange("p (j h) -> p j h", h=H))


_CACHED = {}


def _build(kwin):
    if kwin in _CACHED:
        return _CACHED[kwin]
    KK = kwin + K_RANK
    nc = bacc.Bacc("TRN2", target_bir_lowering=False, debug=False)
    ewin = nc.dram_tensor(
        "ewin", (BPC * (NCHUNK * 2 * H // SEGB) * KK, SEGB), FP8,
        kind="ExternalInput").ap()
    smat = nc.dram_tensor(
        "smat", (BPC * (NCHUNK * 2 * CH // SEGB) * KK, SEGB), FP8,
        kind="ExternalInput").ap()
    out = nc.dram_tensor("out", (BPC * T, H), BF16, kind="ExternalOutput").ap()

    with tile.TileContext(nc) as tc:
        with ExitStack() as ctx:
            _emit(ctx, tc, kwin, ewin, smat, out)
    nc.compile()
    _CACHED[kwin] = nc
    return nc


def _host_prep(encoder_out, pitch, beats, align_phone,
               w_pitch, b_pitch, w_beats, b_beats, w_pos, b_pos):
    """Compute idx on the host; build rank rows, per-chunk windows and
    one-hot/rank lhsT matrices. Returns (kwin, in_maps)."""
    import ml_dtypes
    fp8 = ml_dtypes.float8_e4m3
    f32 = np.float32

    align = np.asarray(align_phone, np.int32)
    change = np.concatenate(
        [np.zeros((B, 1), np.int32),
         (align[:, 1:] != align[:, :-1]).astype(np.int32)], axis=1)
    idx = np.clip(np.cumsum(change, axis=1), 0, P - 1)    # [B, T]

    # chunk spans -> window size (uniform across cores; baked into the NEFF)
    idx_c = idx.reshape(B, NCHUNK, CH)
    lo = idx_c[:, :, 0]                                    # [B, NCHUNK]
    hi = idx_c[:, :, -1]
    span = int((hi - lo + 1).max())
    kwin = max(8, -(-span // 2) + 1)
    kwin = -(-kwin // 4) * 4                               # round up to /4
    assert span <= 2 * kwin
    KK = kwin + K_RANK
    wbase = np.minimum(lo, P - 2 * kwin)                   # [B, NCHUNK]

    # 22 rank rows: lhsT values AR [22, T] (pos digits shared, pitch/beats
    # per batch) and rhs values WR [22, H]
    def e4(x):
        return np.asarray(x, f32).astype(fp8).astype(f32)

    def split3(w):
        w = np.asarray(w, f32)
        s0 = e4(w)
        s1 = e4(w - s0)
        s2 = e4(w - s0 - s1)
        return s0, s1, s2

    def split2(w):
        w = np.asarray(w, f32)
        s0 = e4(w)
        s1 = e4(w - s0)
        return s0, s1

    t = np.arange(T, dtype=np.int64)
    digits = [((t // (8 ** q)) % 8).astype(f32) for q in range(4)]
    ws = split3(w_pos)
    wp = split2(w_pitch)
    wb = split2(w_beats)
    bs = split2(np.asarray(b_pitch, f32) + np.asarray(b_beats, f32)
                + np.asarray(b_pos, f32))
    pit = split2(pitch)                                    # [2][B, T]
    bea = split2(beats)

    WR = np.zeros((22, H), f32)
    AR_shared = np.zeros((22, T), f32)                     # rows 0..11, 20..21
    for q in range(4):
        for s in range(3):
            r = q * 3 + s
            AR_shared[r] = digits[q] * C0[q]
            WR[r] = C1[q] * ws[s]
    WR[12], WR[13], WR[14], WR[15] = wp[0], wp[1], wp[0], wp[1]
    WR[16], WR[17], WR[18], WR[19] = wb[0], wb[1], wb[0], wb[1]
    AR_shared[20] = 1.0
    AR_shared[21] = 1.0
    WR[20], WR[21] = bs[0], bs[1]

    enc = np.ascontiguousarray(encoder_out, f32)           # [B, P, H]

    in_maps = []
    for r in range(NCORES):
        # [KK, bytes-per-partition] images; reshaped to the segment-
        # interleaved DRAM layout ([b, seg, partition, SEGB]) at the end
        ewin_np = np.zeros((KK, BPC * NCHUNK * 2 * H), f32)
        smat_np = np.zeros((KK, BPC * NCHUNK * 2 * CH), f32)
        for bi in range(BPC):
            bg = r * BPC + bi
            AR = AR_shared.copy()
            AR[12], AR[13] = pit[0][bg], pit[0][bg]
            AR[14], AR[15] = pit[1][bg], pit[1][bg]
            AR[16], AR[17] = bea[0][bg], bea[0][bg]
            AR[18], AR[19] = bea[1][bg], bea[1][bg]
            for cl in range(NCHUNK):
                ci = bi * NCHUNK + cl
                w = int(wbase[bg, cl])
                # rhs: window row pairs + rank row pairs
                rows = enc[bg, w:w + 2 * kwin].reshape(kwin, 2 * H)
                ewin_np[:kwin, ci * 2 * H:(ci + 1) * 2 * H] = rows
                ewin_np[kwin:, ci * 2 * H:(ci + 1) * 2 * H] = \
                    WR.reshape(K_RANK, 2 * H)
                # lhsT: one-hot halves + rank rows
                rel = idx[bg, cl * CH:(cl + 1) * CH] - w   # [CH] in [0,2kwin)
                sblk = np.zeros((kwin, 2, CH), f32)
                sblk[rel // 2, rel % 2, np.arange(CH)] = 1.0
                smat_np[:kwin, ci * 2 * CH:(ci + 1) * 2 * CH] = \
                    sblk.reshape(kwin, 2 * CH)
                ablk = AR[:, cl * CH:(cl + 1) * CH].reshape(K_RANK, 2, CH)
                smat_np[kwin:, ci * 2 * CH:(ci + 1) * 2 * CH] = \
                    ablk.reshape(K_RANK, 2 * CH)
        # [KK, BPC*PB] -> [BPC*NSEG*KK, SEGB] with row (b*NSEG+s)*KK+p
        def seg(a):
            pb = a.shape[1] // BPC
            return np.ascontiguousarray(
                a.reshape(KK, BPC, pb // SEGB, SEGB).transpose(1, 2, 0, 3)
                .reshape(-1, SEGB))

        in_maps.append({
            "ewin": seg(ewin_np).astype(fp8),
            "smat": seg(smat_np).astype(fp8),
        })
    return kwin, in_maps


def _run_in_subprocess(kwargs):
    """Fallback for a wedged in-process PJRT client: re-run this module in a
    fresh interpreter (fresh device boot), passing inputs via pickle."""
    import os
    import pickle
    import subprocess
    import tempfile

    with tempfile.TemporaryDirectory() as td:
        inp = os.path.join(td, "in.pkl")
        outp = os.path.join(td, "out.npy")
        with open(inp, "wb") as f:
            pickle.dump(kwargs, f)
        code = (
            "import pickle, numpy as np, importlib.util\n"
            f"spec = importlib.util.spec_from_file_location('k', {__file__!r})\n"
            "m = importlib.util.module_from_spec(spec)\n"
            f"ins = pickle.load(open({inp!r}, 'rb'))\n"
            "spec.loader.exec_module(m)\n"
            f"np.save({outp!r}, m.kernel(**ins, _no_fallback=True))\n"
        )
        subprocess.run([sys.executable, "-c", code], check=True, timeout=1700)
        return np.load(outp)


def kernel(encoder_out, pitch, beats, w_pitch, b_pitch, w_beats, b_beats,
           w_pos, b_pos, align_phone, _trace=False, _no_fallback=False):
    kwargs = dict(encoder_out=np.asarray(encoder_out),
                  pitch=np.asarray(pitch), beats=np.asarray(beats),
                  w_pitch=np.asarray(w_pitch), b_pitch=np.asarray(b_pitch),
                  w_beats=np.asarray(w_beats), b_beats=np.asarray(b_beats),
                  w_pos=np.asarray(w_pos), b_pos=np.asarray(b_pos),
                  align_phone=np.asarray(align_phone))
    kwin, in_maps = _host_prep(
        encoder_out, pitch, beats, align_phone,
        w_pitch, b_pitch, w_beats, b_beats, w_pos, b_pos)
    nc = _build(kwin)

    def attempt():
        # materialize eagerly so device failures surface inside the guard
        res = run_bass_kernel_spmd(nc, in_maps, core_ids=list(range(NCORES)),
                                   trace=_trace)
        return res, np.concatenate(
            [np.asarray(res.results[r]["out"]).astype(np.float32).reshape(
                BPC, T, H) for r in range(NCORES)], axis=0)

    import time
    res = out = None
    for i in range(2):
        try:
            res, out = attempt()
            break
        except Exception:
            # rare flaky device hang (NRT_EXEC_UNIT_UNRECOVERABLE)
            time.sleep(5.0)
    if out is None:
        if _no_fallback:
            res, out = attempt()
        else:
            # fresh interpreter = fresh PJRT client + device reset
            try:
                return _run_in_subprocess(kwargs)
            except Exception:
                time.sleep(10.0)
                return _run_in_subprocess(kwargs)
    if _trace:
        kernel.last_results = res
    return out


# revision 15
# speedup vs baseline: 1.7425x; 1.5615x over previous
"""Trainium2 Bass kernel for nn_Encoder_Postnet (length-regulator gather + per-frame linears).

Contract: kernel(**inputs) takes FULL numpy inputs (as produced by
setup_inputs) and returns the FULL [B, T, H] float32 output. Internally the
batch dim is sharded across 8 NeuronCores (pure data parallel, 4 batches per
core); the tiny Linear(1,H) params are replicated.

v3 design: ONE DoubleRow fp8 matmul per 128-frame chunk computes BOTH the
length-regulator gather and the rank-update linears; ~20.5 MB HBM/core.

  - Per chunk, the encoder rows needed span <= 2*K_WIN consecutive indices
    (idx increments by at most 1 per frame). The host materializes a pair-
    packed window: partition k of the rhs holds rows [w+2k | w+2k+1] (1 KB),
    and 11 extra partitions hold the 22 rank-update rows as pairs.
    DoubleRow matmul semantics (out = lhsT[:,0].T @ rhs[:,0] +
    lhsT[:,1].T @ rhs[:,1], fp8, 2x throughput) then give
      out[m,:] = sum_k S_even[k,m]*row_{w+2k} + S_odd[k,m]*row_{w+2k+1}
               + sum_j A_j[m]*W_j
    with the one-hot S and the A rows host-built in the lhsT. No on-device
    scan, no SWDGE gather (34us of Q7 descriptor-gen on v2), no identity
    matmuls, and PE streams each chunk once at 2 cols/cycle.
  - rank rows (22): pos*w_pos via exact base-8 digit split of t
    (t = sum a_q 8^q, rows a_q*c0_q x c1_q*w_s with c0*c1 = 8^q, all values
    e4m3-exact) against a 3-way e4m3 split of w_pos (residual ~1e-3 rel);
    pitch/beats/bias via hi/lo e4m3 splits.
  - instruction stream is identical across the 8 cores (SPMD NEFF): the
    window base w and one-hots live in the DATA; only the single constant
    K_WIN (from the max chunk span) parameterizes the compiled kernel.
  - finishers: PSUM -> SBUF bf16 copies alternate Scalar/Vector engines
    (GPSIMD has no PSUM port); output written in BF16 (16.8 MB vs 33.5 f32)
    and upcast on the host. Total rel err ~2e-3 vs the 2e-2 gate.
"""

import sys

if "/opt/trn_rl_repo" not in sys.path:
    sys.path.insert(0, "/opt/trn_rl_repo")

from contextlib import ExitStack

import numpy as np

import concourse.bass as bass
import concourse.tile as tile
from concourse import bacc, mybir
from concourse.bass_utils import run_bass_kernel_spmd

B, T, P, H = 32, 4096, 512, 512
NCORES = 8
BPC = B // NCORES            # batches per core
CH = 128                     # frames per chunk (partition dim)
NCHUNK = T // CH             # 32 chunks per batch
GRP = 4                      # chunks per finisher group (4 psum banks)
GRP_T = GRP * CH
NG = NCHUNK // GRP           # 8 groups per batch
K_RANK = 11                  # rank-update row PAIRS (22 rows)
SEGB = 4096                  # input-load DMA segment bytes (descriptor size)
F32 = mybir.dt.float32
BF16 = mybir.dt.bfloat16
FP8 = mybir.dt.float8e4
DR = mybir.MatmulPerfMode.DoubleRow

# pos = t*w_pos with t = sum_q a_q 8^q split as (a_q*c0_q)*(c1_q*w_s):
# every factor exactly representable in e4m3 (<=112 / <=32).
C0 = [1.0, 2.0, 8.0, 16.0]
C1 = [1.0, 4.0, 8.0, 32.0]


def _emit(ctx: ExitStack, tc: tile.TileContext, kwin, ewin, smat, out):
    nc = tc.nc
    KK = kwin + K_RANK
    const = ctx.enter_context(tc.tile_pool(name="const", bufs=1))
    epool = ctx.enter_context(tc.tile_pool(name="epool", bufs=2))
    opool = ctx.enter_context(tc.tile_pool(name="opool", bufs=6))
    ppool = ctx.enter_context(tc.tile_pool(name="ppool", bufs=2, space="PSUM"))

    # A DMA instruction's descriptors drain SERIALLY on a single DMA engine
    # (~22 GB/s); fabric parallelism comes from CONCURRENT DMA instructions.
    # So each input load is split into one dma_start per 4KB segment (23
    # descriptors each); the DRAM side is segment-interleaved (row
    # (b*NSEG+s)*KK+p holds partition p's bytes [s*SEGB,(s+1)*SEGB)).
    EB = NCHUNK * 2 * H          # ewin bytes/partition/batch (32K)
    SB_ = NCHUNK * 2 * CH        # smat bytes/partition/batch (8K)
    NSEG_E = EB // SEGB
    NSEG_S = SB_ // SEGB

    def load_split(dst, src, b, nseg, engs):
        for s in range(nseg):
            engs[s % len(engs)].dma_start(
                dst[:, s * SEGB:(s + 1) * SEGB],
                src[(b * nseg + s) * KK:(b * nseg + s + 1) * KK, :])

    sms = []
    for b in range(BPC):
        sm = const.tile([KK, SB_], FP8, tag=f"sm{b}", name=f"sm{b}")
        sms.append(sm)

    ews = {}

    def load_ewin(b, engs):
        ew = epool.tile([KK, EB], FP8, name=f"ew{b}", tag="ew")
        load_split(ew, ewin, b, NSEG_E, engs)
        ews[b] = ew

    # batch 0 fans out over all three issue rings for the fastest start;
    # later batches go on the otherwise-idle gpsimd ring.
    load_split(sms[0], smat, 0, NSEG_S, [nc.sync, nc.scalar])
    load_ewin(0, [nc.sync, nc.scalar, nc.gpsimd])
    load_split(sms[1], smat, 1, NSEG_S, [nc.gpsimd])
    load_ewin(1, [nc.gpsimd])
    for b in (2, 3):
        load_split(sms[b], smat, b, NSEG_S, [nc.gpsimd])

    for b in range(BPC):
        if b + 2 < BPC:
            load_ewin(b + 2, [nc.gpsimd])
        ew = ews[b]
        sm = sms[b]
        for g in range(NG):
            gi = b * NG + g
            ps = ppool.tile([128, GRP * H], F32)
            for j in range(GRP):
                cl = g * GRP + j
                nc.tensor.matmul(
                    ps[:, j * H:(j + 1) * H],
                    lhsT=sm[:, cl * 2 * CH:(cl + 1) * 2 * CH].rearrange(
                        "p (two m) -> p two m", two=2),
                    rhs=ew[:, cl * 2 * H:(cl + 1) * 2 * H].rearrange(
                        "p (two n) -> p two n", two=2),
                    start=True, stop=True, perf_mode=DR)
            ot = opool.tile([128, GRP * H], BF16)
            if gi % 2 == 0:
                nc.scalar.copy(ot[:], ps[:])
            else:
                nc.vector.tensor_copy(ot[:], ps[:])
            nc.sync.dma_start(
                out[b * T + g * GRP_T: b * T + (g + 1) * GRP_T, :].rearrange(
                    "(j p) h -> p j h", p=128),
                ot[:].rearrange("p (j h) -> p j h", h=H))


_CACHED = {}


def _build(kwin):
    if kwin in _CACHED:
        return _CACHED[kwin]
    KK = kwin + K_RANK
    nc = bacc.Bacc("TRN2", target_bir_lowering=False, debug=False)
    ewin = nc.dram_tensor(
        "ewin", (BPC * (NCHUNK * 2 * H // SEGB) * KK, SEGB), FP8,
        kind="ExternalInput").ap()
    smat = nc.dram_tensor(
        "smat", (BPC * (NCHUNK * 2 * CH // SEGB) * KK, SEGB), FP8,
        kind="ExternalInput").ap()
    out = nc.dram_tensor("out", (BPC * T, H), BF16, kind="ExternalOutput").ap()

    with tile.TileContext(nc) as tc:
        with ExitStack() as ctx:
            _emit(ctx, tc, kwin, ewin, smat, out)
    nc.compile()
    _CACHED[kwin] = nc
    return nc


def _host_prep(encoder_out, pitch, beats, align_phone,
               w_pitch, b_pitch, w_beats, b_beats, w_pos, b_pos):
    """Compute idx on the host; build rank rows, per-chunk windows and
    one-hot/rank lhsT matrices. Returns (kwin, in_maps)."""
    import ml_dtypes
    fp8 = ml_dtypes.float8_e4m3
    f32 = np.float32

    align = np.asarray(align_phone, np.int32)
    change = np.concatenate(
        [np.zeros((B, 1), np.int32),
         (align[:, 1:] != align[:, :-1]).astype(np.int32)], axis=1)
    idx = np.clip(np.cumsum(change, axis=1), 0, P - 1)    # [B, T]

    # chunk spans -> window size (uniform across cores; baked into the NEFF)
    idx_c = idx.reshape(B, NCHUNK, CH)
    lo = idx_c[:, :, 0]                                    # [B, NCHUNK]
    hi = idx_c[:, :, -1]
    span = int((hi - lo + 1).max())
    kwin = max(8, -(-span // 2) + 1)
    kwin = -(-kwin // 4) * 4                               # round up to /4
    assert span <= 2 * kwin
    KK = kwin + K_RANK
    wbase = np.minimum(lo, P - 2 * kwin)                   # [B, NCHUNK]

    # 22 rank rows: lhsT values AR [22, T] (pos digits shared, pitch/beats
    # per batch) and rhs values WR [22, H]
    def e4(x):
        return np.asarray(x, f32).astype(fp8).astype(f32)

    def split3(w):
        w = np.asarray(w, f32)
        s0 = e4(w)
        s1 = e4(w - s0)
        s2 = e4(w - s0 - s1)
        return s0, s1, s2

    def split2(w):
        w = np.asarray(w, f32)
        s0 = e4(w)
        s1 = e4(w - s0)
        return s0, s1

    t = np.arange(T, dtype=np.int64)
    digits = [((t // (8 ** q)) % 8).astype(f32) for q in range(4)]
    ws = split3(w_pos)
    wp = split2(w_pitch)
    wb = split2(w_beats)
    bs = split2(np.asarray(b_pitch, f32) + np.asarray(b_beats, f32)
                + np.asarray(b_pos, f32))
    pit = split2(pitch)                                    # [2][B, T]
    bea = split2(beats)

    WR = np.zeros((22, H), f32)
    AR_shared = np.zeros((22, T), f32)                     # rows 0..11, 20..21
    for q in range(4):
        for s in range(3):
            r = q * 3 + s
            AR_shared[r] = digits[q] * C0[q]
            WR[r] = C1[q] * ws[s]
    WR[12], WR[13], WR[14], WR[15] = wp[0], wp[1], wp[0], wp[1]
    WR[16], WR[17], WR[18], WR[19] = wb[0], wb[1], wb[0], wb[1]
    AR_shared[20] = 1.0
    AR_shared[21] = 1.0
    WR[20], WR[21] = bs[0], bs[1]

    enc = np.ascontiguousarray(encoder_out, f32)           # [B, P, H]

    in_maps = []
    for r in range(NCORES):
        # [KK, bytes-per-partition] images; reshaped to the segment-
        # interleaved DRAM layout ([b, seg, partition, SEGB]) at the end
        ewin_np = np.zeros((KK, BPC * NCHUNK * 2 * H), f32)
        smat_np = np.zeros((KK, BPC * NCHUNK * 2 * CH), f32)
        for bi in range(BPC):
            bg = r * BPC + bi
            AR = AR_shared.copy()
            AR[12], AR[13] = pit[0][bg], pit[0][bg]
            AR[14], AR[15] = pit[1][bg], pit[1][bg]
            AR[16], AR[17] = bea[0][bg], bea[0][bg]
            AR[18], AR[19] = bea[1][bg], bea[1][bg]
            for cl in range(NCHUNK):
                ci = bi * NCHUNK + cl
                w = int(wbase[bg, cl])
                # rhs: window row pairs + rank row pairs
                rows = enc[bg, w:w + 2 * kwin].reshape(kwin, 2 * H)
                ewin_np[:kwin, ci * 2 * H:(ci + 1) * 2 * H] = rows
                ewin_np[kwin:, ci * 2 * H:(ci + 1) * 2 * H] = \
                    WR.reshape(K_RANK, 2 * H)
                # lhsT: one-hot halves + rank rows
                rel = idx[bg, cl * CH:(cl + 1) * CH] - w   # [CH] in [0,2kwin)
                sblk = np.zeros((kwin, 2, CH), f32)
                sblk[rel // 2, rel % 2, np.arange(CH)] = 1.0
                smat_np[:kwin, ci * 2 * CH:(ci + 1) * 2 * CH] = \
                    sblk.reshape(kwin, 2 * CH)
                ablk = AR[:, cl * CH:(cl + 1) * CH].reshape(K_RANK, 2, CH)
                smat_np[kwin:, ci * 2 * CH:(ci + 1) * 2 * CH] = \
                    ablk.reshape(K_RANK, 2 * CH)
        # [KK, BPC*PB] -> [BPC*NSEG*KK, SEGB] with row (b*NSEG+s)*KK+p
        def seg(a):
            pb = a.shape[1] // BPC
            return np.ascontiguousarray(
                a.reshape(KK, BPC, pb // SEGB, SEGB).transpose(1, 2, 0, 3)
                .reshape(-1, SEGB))

        in_maps.append({
            "ewin": seg(ewin_np).astype(fp8),
            "smat": seg(smat_np).astype(fp8),
        })
    return kwin, in_maps


def _run_in_subprocess(kwargs):
    """Fallback for a wedged in-process PJRT client: re-run this module in a
    fresh interpreter (fresh device boot), passing inputs via pickle."""
    import os
    import pickle
    import subprocess
    import tempfile

    with tempfile.TemporaryDirectory() as td:
        inp = os.path.join(td, "in.pkl")
        outp = os.path.join(td, "out.npy")
        with open(inp, "wb") as f:
            pickle.dump(kwargs, f)
        code = (
            "import pickle, numpy as np, importlib.util\n"
            f"spec = importlib.util.spec_from_file_location('k', {__file__!r})\n"
            "m = importlib.util.module_from_spec(spec)\n"
            f"ins = pickle.load(open({inp!r}, 'rb'))\n"
            "spec.loader.exec_module(m)\n"
            f"np.save({outp!r}, m.kernel(**ins, _no_fallback=True))\n"
        )
        subprocess.run([sys.executable, "-c", code], check=True, timeout=1700)
        return np.load(outp)


def kernel(encoder_out, pitch, beats, w_pitch, b_pitch, w_beats, b_beats,
           w_pos, b_pos, align_phone, _trace=False, _no_fallback=False):
    kwargs = dict(encoder_out=np.asarray(encoder_out),
                  pitch=np.asarray(pitch), beats=np.asarray(beats),
                  w_pitch=np.asarray(w_pitch), b_pitch=np.asarray(b_pitch),
                  w_beats=np.asarray(w_beats), b_beats=np.asarray(b_beats),
                  w_pos=np.asarray(w_pos), b_pos=np.asarray(b_pos),
                  align_phone=np.asarray(align_phone))
    kwin, in_maps = _host_prep(
        encoder_out, pitch, beats, align_phone,
        w_pitch, b_pitch, w_beats, b_beats, w_pos, b_pos)
    nc = _build(kwin)

    def attempt():
        # materialize eagerly so device failures surface inside the guard
        res = run_bass_kernel_spmd(nc, in_maps, core_ids=list(range(NCORES)),
                                   trace=_trace)
        return res, np.concatenate(
            [np.asarray(res.results[r]["out"]).astype(np.float32).reshape(
                BPC, T, H) for r in range(NCORES)], axis=0)

    import time
    res = out = None
    for i in range(2):
        try:
            res, out = attempt()
            break
        except Exception:
            # rare flaky device hang (NRT_EXEC_UNIT_UNRECOVERABLE)
            time.sleep(5.0)
    if out is None:
        if _no_fallback:
            res, out = attempt()
        else:
            # fresh interpreter = fresh PJRT client + device reset
            try:
                return _run_in_subprocess(kwargs)
            except Exception:
                time.sleep(10.0)
                return _run_in_subprocess(kwargs)
    if _trace:
        kernel.last_results = res
    return out


# revision 17
# speedup vs baseline: 2.2200x; 1.2740x over previous
"""Trainium2 Bass kernel for nn_Encoder_Postnet (length-regulator gather + per-frame linears).

Contract: kernel(**inputs) takes FULL numpy inputs (as produced by
setup_inputs) and returns the FULL [B, T, H] float32 output. Internally the
batch dim is sharded across 8 NeuronCores (pure data parallel, 4 batches per
core); the tiny Linear(1,H) params are replicated.

v3 design: ONE DoubleRow fp8 matmul per 128-frame chunk computes BOTH the
length-regulator gather and the rank-update linears; ~20.5 MB HBM/core.

  - Per chunk, the encoder rows needed span <= 2*K_WIN consecutive indices
    (idx increments by at most 1 per frame). The host materializes a pair-
    packed window: partition k of the rhs holds rows [w+2k | w+2k+1] (1 KB),
    and 11 extra partitions hold the 22 rank-update rows as pairs.
    DoubleRow matmul semantics (out = lhsT[:,0].T @ rhs[:,0] +
    lhsT[:,1].T @ rhs[:,1], fp8, 2x throughput) then give
      out[m,:] = sum_k S_even[k,m]*row_{w+2k} + S_odd[k,m]*row_{w+2k+1}
               + sum_j A_j[m]*W_j
    with the one-hot S and the A rows host-built in the lhsT. No on-device
    scan, no SWDGE gather (34us of Q7 descriptor-gen on v2), no identity
    matmuls, and PE streams each chunk once at 2 cols/cycle.
  - rank rows (22): pos*w_pos via exact base-8 digit split of t
    (t = sum a_q 8^q, rows a_q*c0_q x c1_q*w_s with c0*c1 = 8^q, all values
    e4m3-exact) against a 3-way e4m3 split of w_pos (residual ~1e-3 rel);
    pitch/beats/bias via hi/lo e4m3 splits.
  - instruction stream is identical across the 8 cores (SPMD NEFF): the
    window base w and one-hots live in the DATA; only the single constant
    K_WIN (from the max chunk span) parameterizes the compiled kernel.
  - finishers: PSUM -> SBUF bf16 copies alternate Scalar/Vector engines
    (GPSIMD has no PSUM port); output written in BF16 (16.8 MB vs 33.5 f32)
    and upcast on the host. Total rel err ~2e-3 vs the 2e-2 gate.
"""

import sys

if "/opt/trn_rl_repo" not in sys.path:
    sys.path.insert(0, "/opt/trn_rl_repo")

from contextlib import ExitStack

import numpy as np

import concourse.bass as bass
import concourse.tile as tile
from concourse import bacc, mybir
from concourse.bass_utils import run_bass_kernel_spmd

B, T, P, H = 32, 4096, 512, 512
NCORES = 8
BPC = B // NCORES            # batches per core
CH = 128                     # frames per chunk (partition dim)
NCHUNK = T // CH             # 32 chunks per batch
GRP = 4                      # chunks per finisher group (4 psum banks)
GRP_T = GRP * CH
NG = NCHUNK // GRP           # 8 groups per batch
K_RANK = 11                  # rank-update row PAIRS (22 rows)
SEGB = 4096                  # input-load DMA segment bytes (descriptor size)
F32 = mybir.dt.float32
BF16 = mybir.dt.bfloat16
FP8 = mybir.dt.float8e4
DR = mybir.MatmulPerfMode.DoubleRow

# pos = t*w_pos with t = sum_q a_q 8^q split as (a_q*c0_q)*(c1_q*w_s):
# every factor exactly representable in e4m3 (<=112 / <=32).
C0 = [1.0, 2.0, 8.0, 16.0]
C1 = [1.0, 4.0, 8.0, 32.0]


def _emit(ctx: ExitStack, tc: tile.TileContext, kwin, ewin, smat, out):
    nc = tc.nc
    KK = kwin + K_RANK
    const = ctx.enter_context(tc.tile_pool(name="const", bufs=1))
    epool = ctx.enter_context(tc.tile_pool(name="epool", bufs=2))
    opool = ctx.enter_context(tc.tile_pool(name="opool", bufs=8))
    ppool = ctx.enter_context(tc.tile_pool(name="ppool", bufs=4, space="PSUM"))

    # A DMA instruction's descriptors drain SERIALLY on a single DMA engine
    # (~22 GB/s) when the SBUF side has < 128 partitions; the gpsimd ring
    # round-robins instructions across engines, so every input load goes
    # there as one dma_start per 4KB segment (= one 4-chunk group of rhs
    # data; 23 descriptors each). The DRAM side is segment-interleaved (row
    # (b*NSEG+s)*KK+p holds partition p's bytes [s*SEGB,(s+1)*SEGB)).
    # SBUF->DRAM writes DO spread one instruction's descriptors over all 16
    # engines, so the sync ring keeps the big output writes.
    EB = NCHUNK * 2 * H          # ewin bytes/partition/batch (32K)
    SB_ = NCHUNK * 2 * CH        # smat bytes/partition/batch (8K)
    NSEG_E = EB // SEGB
    NSEG_S = SB_ // SEGB

    def load_seg(dst, src, b, s, nseg):
        nc.gpsimd.dma_start(
            dst[:, s * SEGB:(s + 1) * SEGB],
            src[(b * nseg + s) * KK:(b * nseg + s + 1) * KK, :])

    sms = []
    for b in range(BPC):
        sm = const.tile([KK, SB_], FP8, tag=f"sm{b}", name=f"sm{b}")
        sms.append(sm)

    ews = {}

    def load_ewin(b):
        ew = epool.tile([KK, EB], FP8, name=f"ew{b}", tag="ew")
        # in consumption order: lhsT seg (2 groups) before its rhs segs
        for s in range(NSEG_E):
            if s % (NSEG_E // NSEG_S) == 0:
                load_seg(sms[b], smat, b, s // (NSEG_E // NSEG_S), NSEG_S)
            load_seg(ew, ewin, b, s, NSEG_E)
        ews[b] = ew

    load_ewin(0)
    load_ewin(1)

    # 2-chunk psum groups, 4 in flight: shorter PE stalls at group
    # boundaries than 2x4-chunk ping-pong
    PG = 2
    for b in range(BPC):
        if b + 2 < BPC:
            load_ewin(b + 2)
        ew = ews[b]
        sm = sms[b]
        for g in range(NCHUNK // PG):
            gi = b * (NCHUNK // PG) + g
            ps = ppool.tile([128, PG * H], F32)
            for j in range(PG):
                cl = g * PG + j
                nc.tensor.matmul(
                    ps[:, j * H:(j + 1) * H],
                    lhsT=sm[:, cl * 2 * CH:(cl + 1) * 2 * CH].rearrange(
                        "p (two m) -> p two m", two=2),
                    rhs=ew[:, cl * 2 * H:(cl + 1) * 2 * H].rearrange(
                        "p (two n) -> p two n", two=2),
                    start=True, stop=True, perf_mode=DR)
            ot = opool.tile([128, PG * H], BF16)
            if gi % 2 == 0:
                nc.scalar.copy(ot[:], ps[:])
            else:
                nc.vector.tensor_copy(ot[:], ps[:])
            nc.sync.dma_start(
                out[b * T + g * PG * CH: b * T + (g + 1) * PG * CH,
                    :].rearrange("(j p) h -> p j h", p=128),
                ot[:].rearrange("p (j h) -> p j h", h=H))


_CACHED = {}


def _build(kwin):
    if kwin in _CACHED:
        return _CACHED[kwin]
    KK = kwin + K_RANK
    nc = bacc.Bacc("TRN2", target_bir_lowering=False, debug=False)
    ewin = nc.dram_tensor(
        "ewin", (BPC * (NCHUNK * 2 * H // SEGB) * KK, SEGB), FP8,
        kind="ExternalInput").ap()
    smat = nc.dram_tensor(
        "smat", (BPC * (NCHUNK * 2 * CH // SEGB) * KK, SEGB), FP8,
        kind="ExternalInput").ap()
    out = nc.dram_tensor("out", (BPC * T, H), BF16, kind="ExternalOutput").ap()

    with tile.TileContext(nc) as tc:
        with ExitStack() as ctx:
            _emit(ctx, tc, kwin, ewin, smat, out)
    nc.compile()
    _CACHED[kwin] = nc
    return nc


def _host_prep(encoder_out, pitch, beats, align_phone,
               w_pitch, b_pitch, w_beats, b_beats, w_pos, b_pos):
    """Compute idx on the host; build rank rows, per-chunk windows and
    one-hot/rank lhsT matrices. Returns (kwin, in_maps)."""
    import ml_dtypes
    fp8 = ml_dtypes.float8_e4m3
    f32 = np.float32

    align = np.asarray(align_phone, np.int32)
    change = np.concatenate(
        [np.zeros((B, 1), np.int32),
         (align[:, 1:] != align[:, :-1]).astype(np.int32)], axis=1)
    idx = np.clip(np.cumsum(change, axis=1), 0, P - 1)    # [B, T]

    # chunk spans -> window size (uniform across cores; baked into the NEFF)
    idx_c = idx.reshape(B, NCHUNK, CH)
    lo = idx_c[:, :, 0]                                    # [B, NCHUNK]
    hi = idx_c[:, :, -1]
    span = int((hi - lo + 1).max())
    kwin = max(8, -(-span // 2) + 1)
    kwin = -(-kwin // 4) * 4                               # round up to /4
    assert span <= 2 * kwin
    KK = kwin + K_RANK
    wbase = np.minimum(lo, P - 2 * kwin)                   # [B, NCHUNK]

    # 22 rank rows: lhsT values AR [22, T] (pos digits shared, pitch/beats
    # per batch) and rhs values WR [22, H]
    def e4(x):
        return np.asarray(x, f32).astype(fp8).astype(f32)

    def split3(w):
        w = np.asarray(w, f32)
        s0 = e4(w)
        s1 = e4(w - s0)
        s2 = e4(w - s0 - s1)
        return s0, s1, s2

    def split2(w):
        w = np.asarray(w, f32)
        s0 = e4(w)
        s1 = e4(w - s0)
        return s0, s1

    t = np.arange(T, dtype=np.int64)
    digits = [((t // (8 ** q)) % 8).astype(f32) for q in range(4)]
    ws = split3(w_pos)
    wp = split2(w_pitch)
    wb = split2(w_beats)
    bs = split2(np.asarray(b_pitch, f32) + np.asarray(b_beats, f32)
                + np.asarray(b_pos, f32))
    pit = split2(pitch)                                    # [2][B, T]
    bea = split2(beats)

    WR = np.zeros((22, H), f32)
    AR_shared = np.zeros((22, T), f32)                     # rows 0..11, 20..21
    for q in range(4):
        for s in range(3):
            r = q * 3 + s
            AR_shared[r] = digits[q] * C0[q]
            WR[r] = C1[q] * ws[s]
    WR[12], WR[13], WR[14], WR[15] = wp[0], wp[1], wp[0], wp[1]
    WR[16], WR[17], WR[18], WR[19] = wb[0], wb[1], wb[0], wb[1]
    AR_shared[20] = 1.0
    AR_shared[21] = 1.0
    WR[20], WR[21] = bs[0], bs[1]

    enc = np.ascontiguousarray(encoder_out, f32)           # [B, P, H]

    in_maps = []
    for r in range(NCORES):
        # [KK, bytes-per-partition] images; reshaped to the segment-
        # interleaved DRAM layout ([b, seg, partition, SEGB]) at the end
        ewin_np = np.zeros((KK, BPC * NCHUNK * 2 * H), f32)
        smat_np = np.zeros((KK, BPC * NCHUNK * 2 * CH), f32)
        for bi in range(BPC):
            bg = r * BPC + bi
            AR = AR_shared.copy()
            AR[12], AR[13] = pit[0][bg], pit[0][bg]
            AR[14], AR[15] = pit[1][bg], pit[1][bg]
            AR[16], AR[17] = bea[0][bg], bea[0][bg]
            AR[18], AR[19] = bea[1][bg], bea[1][bg]
            for cl in range(NCHUNK):
                ci = bi * NCHUNK + cl
                w = int(wbase[bg, cl])
                # rhs: window row pairs + rank row pairs
                rows = enc[bg, w:w + 2 * kwin].reshape(kwin, 2 * H)
                ewin_np[:kwin, ci * 2 * H:(ci + 1) * 2 * H] = rows
                ewin_np[kwin:, ci * 2 * H:(ci + 1) * 2 * H] = \
                    WR.reshape(K_RANK, 2 * H)
                # lhsT: one-hot halves + rank rows
                rel = idx[bg, cl * CH:(cl + 1) * CH] - w   # [CH] in [0,2kwin)
                sblk = np.zeros((kwin, 2, CH), f32)
                sblk[rel // 2, rel % 2, np.arange(CH)] = 1.0
                smat_np[:kwin, ci * 2 * CH:(ci + 1) * 2 * CH] = \
                    sblk.reshape(kwin, 2 * CH)
                ablk = AR[:, cl * CH:(cl + 1) * CH].reshape(K_RANK, 2, CH)
                smat_np[kwin:, ci * 2 * CH:(ci + 1) * 2 * CH] = \
                    ablk.reshape(K_RANK, 2 * CH)
        # [KK, BPC*PB] -> [BPC*NSEG*KK, SEGB] with row (b*NSEG+s)*KK+p
        def seg(a):
            pb = a.shape[1] // BPC
            return np.ascontiguousarray(
                a.reshape(KK, BPC, pb // SEGB, SEGB).transpose(1, 2, 0, 3)
                .reshape(-1, SEGB))

        in_maps.append({
            "ewin": seg(ewin_np).astype(fp8),
            "smat": seg(smat_np).astype(fp8),
        })
    return kwin, in_maps


def _run_in_subprocess(kwargs):
    """Fallback for a wedged in-process PJRT client: re-run this module in a
    fresh interpreter (fresh device boot), passing inputs via pickle."""
    import os
    import pickle
    import subprocess
    import tempfile

    with tempfile.TemporaryDirectory() as td:
        inp = os.path.join(td, "in.pkl")
        outp = os.path.join(td, "out.npy")
        with open(inp, "wb") as f:
            pickle.dump(kwargs, f)
        code = (
            "import pickle, numpy as np, importlib.util\n"
            f"spec = importlib.util.spec_from_file_location('k', {__file__!r})\n"
            "m = importlib.util.module_from_spec(spec)\n"
            f"ins = pickle.load(open({inp!r}, 'rb'))\n"
            "spec.loader.exec_module(m)\n"
            f"np.save({outp!r}, m.kernel(**ins, _no_fallback=True))\n"
        )
        subprocess.run([sys.executable, "-c", code], check=True, timeout=1700)
        return np.load(outp)


def kernel(encoder_out, pitch, beats, w_pitch, b_pitch, w_beats, b_beats,
           w_pos, b_pos, align_phone, _trace=False, _no_fallback=False):
    kwargs = dict(encoder_out=np.asarray(encoder_out),
                  pitch=np.asarray(pitch), beats=np.asarray(beats),
                  w_pitch=np.asarray(w_pitch), b_pitch=np.asarray(b_pitch),
                  w_beats=np.asarray(w_beats), b_beats=np.asarray(b_beats),
                  w_pos=np.asarray(w_pos), b_pos=np.asarray(b_pos),
                  align_phone=np.asarray(align_phone))
    kwin, in_maps = _host_prep(
        encoder_out, pitch, beats, align_phone,
        w_pitch, b_pitch, w_beats, b_beats, w_pos, b_pos)
    nc = _build(kwin)

    def attempt():
        # materialize eagerly so device failures surface inside the guard
        res = run_bass_kernel_spmd(nc, in_maps, core_ids=list(range(NCORES)),
                                   trace=_trace)
        return res, np.concatenate(
            [np.asarray(res.results[r]["out"]).astype(np.float32).reshape(
                BPC, T, H) for r in range(NCORES)], axis=0)

    import time
    res = out = None
    for i in range(2):
        try:
            res, out = attempt()
            break
        except Exception:
            # rare flaky device hang (NRT_EXEC_UNIT_UNRECOVERABLE)
            time.sleep(5.0)
    if out is None:
        if _no_fallback:
            res, out = attempt()
        else:
            # fresh interpreter = fresh PJRT client + device reset
            try:
                return _run_in_subprocess(kwargs)
            except Exception:
                time.sleep(10.0)
                return _run_in_subprocess(kwargs)
    if _trace:
        kernel.last_results = res
    return out
